# revision 20
# baseline (speedup 1.0000x reference)
"""Trainium2 Bass kernel for nn_Attention_68401649156342.

Reference computation (per batch element b of 8):
    q = MLP_q(x[b])                 # [2048,128] -> relu(x@Wq1+bq1)@Wq2+bq2 -> [2048,256]
    k = MLP_k(x[b])
    s = q @ k.T                     # [2048,2048]
    m = rowmax(s)
    out[b] = softmax(s / m, axis=-1)

Sharding: pure data-parallel over batch. Each of the 8 NeuronCores handles one
batch element end-to-end; no collectives.

Per-core dataflow (bf16 compute, f32 accumulate). The scores+softmax loop is
engine-balanced (ACT is the floor at ~2.4us/tile):
    - host pre-transposes x[b] -> xT [128(F),2048(S)], casts x/W to bf16, and
      packs biases + the exp bias constant into one [128,9] f32 block
    - xT is DMAed in 4 column chunks so layer-1 matmuls start ~1us in; a few
      dummy matmuls run during the DMA wait to open the PE clock gate
    - MLP layer 1: hT[d,s] = relu(W1.T @ xT + b1)  (PE matmul; relu+bias as one
      ACT pass per 1024-col half)
    - MLP layer 2: qT/kT[d,s] = W2.T @ hT + b2     (PE matmul; bias-add epilogue
      split DVE/POOL; K side whole-halves, Q side per-512-chunk so the first
      scores tiles unblock before Q is fully done)
    - scores tile m (16x): 8 matmuls (4 chunks x 2 ktiles) into a [128,2048]
      f32 PSUM tile (4 banks, bufs=2)
    - row-max as two partial maxes OVERLAPPED with the matmul chunks:
      POOL reduces cols 0:1280 (after chunk 2), DVE reduces cols 1280:2048
      (after chunk 3); tiny DVE combine + reciprocal
    - ScalarE: p = exp(scores * (1/max) - 1) in two 1024-col halves, each
      PSUM->SBUF bf16 with fused row-sum accumulation; half-granularity frees
      PSUM banks early so matmul m+2 can start after half A
    - VectorE: rowsum combine + reciprocal, then o = p * (1/sum) (bf16 4x mode)
    - DMA o (bf16) -> out[b]; host upcasts to f32
"""

import os
from contextlib import ExitStack

import ml_dtypes
import numpy as np

B, S, F, D = 8, 2048, 128, 256
NCORES = 8

_CACHED = {}


def _build():
    import concourse.bass as bass
    import concourse.tile as tile
    from concourse import bacc, mybir

    f32 = mybir.dt.float32
    bf16 = mybir.dt.bfloat16
    AF = mybir.ActivationFunctionType
    OP = mybir.AluOpType
    AX = mybir.AxisListType

    nc = bacc.Bacc("TRN2", target_bir_lowering=False, debug=False,
                   num_devices=NCORES)

    xT_d = nc.dram_tensor("xT", [F, S], bf16, kind="ExternalInput")
    w1_d = nc.dram_tensor("W1", [2, F, D], bf16, kind="ExternalInput")
    # W2 pre-tiled on host: [side, ktile, 128, D]
    w2_d = nc.dram_tensor("W2", [2, 2, 128, D], bf16, kind="ExternalInput")
    # host-packed per-partition constants: cols 0-3 = b1[s][m], 4-7 = b2[s][m2],
    # col 8 = -1.0 (exp bias)
    bc_d = nc.dram_tensor("BC", [128, 9], f32, kind="ExternalInput")
    out_d = nc.dram_tensor("out", [S, S], bf16, kind="ExternalOutput")

    NT = S // 128   # 16 score row-tiles
    NCH = S // 512  # 4 free-dim chunks per 2048 span
    H = S // 2      # 1024-col half
    PB = 1280       # POOL/DVE partial-max column split

    with tile.TileContext(nc) as tc, ExitStack() as ctx:
        persist = ctx.enter_context(tc.tile_pool(name="persist", bufs=1))
        hpool = ctx.enter_context(tc.tile_pool(name="hpool", bufs=1))
        psum = ctx.enter_context(
            tc.tile_pool(name="psum", bufs=2, space="PSUM"))
        ppool = ctx.enter_context(tc.tile_pool(name="ppool", bufs=3))
        opool = ctx.enter_context(tc.tile_pool(name="opool", bufs=3))
        stats = ctx.enter_context(tc.tile_pool(name="stats", bufs=4))

        # ---- constant / persistent loads ----
        # Input DMAs are spread across three DGE queues (SP, DVE, ACT) so
        # their ~0.65us descriptor-generation costs run in parallel: xT
        # chunks on SP (first matmul needs xT c0 + w1 ASAP), w1+bc on DVE,
        # w2 (needed ~4us later) on ACT.
        bc = persist.tile([128, 9], f32, tag="bc")

        def b1sb(s, m):
            return bc[:, 2 * s + m:2 * s + m + 1]

        def b2sb(s, m2):
            return bc[:, 4 + 2 * s + m2:4 + 2 * s + m2 + 1]

        neg1 = bc[:, 8:9]

        w1 = persist.tile([F, 2, D], bf16, tag="w1")
        nc.scalar.dma_start(w1[:], w1_d.ap().rearrange("s p d -> p s d"))
        nc.scalar.dma_start(bc[:], bc_d[:])

        xT = persist.tile([F, S], bf16, tag="xT")
        for n in range(NCH):
            nc.sync.dma_start(xT[:, n * 512:(n + 1) * 512],
                              xT_d[:, n * 512:(n + 1) * 512])

        w2 = persist.tile([128, 2, 2, D], bf16, tag="w2")
        nc.scalar.dma_start(w2[:], w2_d.ap().rearrange("s k p d -> p s k d"))

        # ---- PE warm-up: narrow dummy matmuls (128 cols, ~110ns each) run
        # during the input-DMA wait to open the HAM clock-gate without
        # delaying the first real matmul ----
        warm = persist.tile([128, 128], bf16, tag="warm")
        nc.gpsimd.memset(warm[:], 0.0)
        wps = psum.tile([128, S], f32, tag="ps", name="wps")
        for i in range(10):
            nc.tensor.matmul(wps[:, 0:128], warm[:], warm[:],
                             start=True, stop=True)

        # ---- MLPs: produce qT/kT [2][128, S] bf16 (partition = feature d) ----
        # K side (s=1) first: every scores tile needs the full kT, while qT is
        # consumed in 128-col slices (tile m needs q cols [128m,128m+128)).
        qk = [[None, None], [None, None]]  # [side][dtile]
        for s in (1, 0):  # k side, then q side
            # L1 matmuls for both d-tiles (2 PSUM slots), then relu halves
            # interleaved (m0h0, m1h0, m0h1, m1h1) so L2 chunks 0-1 unblock
            # after two ACT passes.
            ps1 = [None, None]
            for m in range(2):
                ps1[m] = psum.tile([128, S], f32, tag="ps", name=f"ps1_{s}_{m}")
                for n in range(NCH):
                    nc.tensor.matmul(
                        ps1[m][:, n * 512:(n + 1) * 512],
                        w1[:, s, m * 128:(m + 1) * 128],
                        xT[:, n * 512:(n + 1) * 512],
                        start=True, stop=True)
            h = [None, None]
            for m in range(2):
                h[m] = hpool.tile([128, S], bf16, tag=f"h{m}", name=f"h_{s}_{m}")
            for qq in range(NCH):
                # relu(ps + b1) -> bf16 in 512-col quarters: d-tile 0 on ACT,
                # d-tile 1 on DVE in parallel; quarter n unblocks L2 chunk n
                sl = slice(qq * 512, (qq + 1) * 512)
                nc.scalar.activation(h[0][:, sl], ps1[0][:, sl],
                                     AF.Relu, bias=b1sb(s, 0), scale=1.0)
                nc.vector.tensor_scalar(h[1][:, sl], ps1[1][:, sl],
                                        b1sb(s, 1), 0.0,
                                        op0=OP.add, op1=OP.max)

            ps2 = [None, None]
            for m2 in range(2):
                ps2[m2] = psum.tile([128, S], f32, tag="ps", name=f"ps2_{s}_{m2}")
                qk[s][m2] = persist.tile([128, S], bf16, tag=f"qk{s}{m2}",
                                         name=f"qk_{s}_{m2}")
            # chunk-major over both d-tiles so epilogue chunk n (and for Q,
            # scores tiles 4n..4n+3) unblocks as early as possible
            for n in range(NCH):
                for m2 in range(2):
                    for k2 in range(2):
                        nc.tensor.matmul(
                            ps2[m2][:, n * 512:(n + 1) * 512],
                            w2[:, s, k2, m2 * 128:(m2 + 1) * 128],
                            h[k2][:, n * 512:(n + 1) * 512],
                            start=(k2 == 0), stop=(k2 == 1))
            if s == 1:
                # K epilogue: bias add in halves; m2=0 on ACT (Identity+bias),
                # m2=1 on DVE, so both d-tiles epilogue in parallel
                for hh in range(2):
                    nc.scalar.activation(qk[s][0][:, hh * H:(hh + 1) * H],
                                         ps2[0][:, hh * H:(hh + 1) * H],
                                         AF.Identity, bias=b2sb(s, 0),
                                         scale=1.0)
                    nc.vector.tensor_scalar_add(qk[s][1][:, hh * H:(hh + 1) * H],
                                                ps2[1][:, hh * H:(hh + 1) * H],
                                                b2sb(s, 1))
            else:
                # Q epilogue: per-512-chunk adds, chunk-major so scores tiles
                # 0-3 unblock after the first pair; m2=0 ACT, m2=1 DVE
                for n in range(NCH):
                    nc.scalar.activation(
                        qk[s][0][:, n * 512:(n + 1) * 512],
                        ps2[0][:, n * 512:(n + 1) * 512],
                        AF.Identity, bias=b2sb(s, 0), scale=1.0)
                    nc.vector.tensor_scalar_add(
                        qk[s][1][:, n * 512:(n + 1) * 512],
                        ps2[1][:, n * 512:(n + 1) * 512], b2sb(s, 1))

        q, k = qk[0], qk[1]

        # ---- scores + softmax, tile by tile ----
        # Engine split (hard constraints: POOL can't touch PSUM; only one
        # PSUM input per instruction; ACT has no max; DVE reduce has no fast
        # modes): DVE owns MAX + recips + MULT (~3.4us), ACT owns EXP+accum
        # (~2.3us), PE ~2us. DVE is the floor; avoid any op that couples
        # ACT's queue to DVE results from a later tile.
        # Software-pipelined: tile m's head (matmul, MAX, r, EXP) is emitted
        # BEFORE tile m-1's tail (rs, MULT, DMA), so in the DVE queue r(m)
        # precedes rs(m-1)/MULT(m-1). Otherwise the scheduler runs the
        # previous tile's tail between MAX(m) and r(m), delaying EXP(m) by
        # ~0.9us — which delays the PSUM release, idles the PE, and drops
        # its p-state to half clock.
        tail = None  # (m, p, ssum)

        def emit_tail(t):
            tm, tp, tssum = t
            rs = stats.tile([128, 1], f32, tag="rs", name=f"rs_{tm}")
            nc.vector.reciprocal(rs[:], tssum[:])
            o = opool.tile([128, S], bf16, tag="o", name=f"o_{tm}")
            # o = p * (1/sum)  [DVE 4x bf16]
            nc.vector.tensor_scalar_mul(o[:], tp[:], rs[:])
            nc.sync.dma_start(out_d[tm * 128:(tm + 1) * 128, :], o[:])

        for m in range(NT):
            ps = psum.tile([128, S], f32, tag="ps")
            for kk in range(2):
                for n in range(NCH):
                    nc.tensor.matmul(
                        ps[:, n * 512:(n + 1) * 512],
                        q[kk][:, m * 128:(m + 1) * 128],
                        k[kk][:, n * 512:(n + 1) * 512],
                        start=(kk == 0), stop=(kk == 1))

            # mx has bufs=1: the next tile's reduce can only claim the slot
            # after this tile's reciprocal consumed mx, which forces the
            # scheduler to place each reciprocal right after its own reduce
            mx = stats.tile([128, 1], f32, tag="mx", bufs=1)
            r = stats.tile([128, 1], f32, tag="r")
            nc.vector.reduce_max(mx[:], ps[:], axis=AX.X)
            nc.vector.reciprocal(r[:], mx[:])
            p = ppool.tile([128, S], bf16, tag="p")
            ssum = stats.tile([128, 1], f32, tag="ssum")
            # p = exp(ps*(1/mx) - 1), ssum = rowsum(p) [one ACT pass]
            nc.scalar.activation(p[:], ps[:], AF.Exp,
                                 bias=neg1, scale=r[:], accum_out=ssum[:])
            if tail is not None:
                emit_tail(tail)
            tail = (m, p, ssum)
        emit_tail(tail)

    nc.compile()
    return nc


def _get_nc():
    if "nc" not in _CACHED:
        _CACHED["nc"] = _build()
    return _CACHED["nc"]


def _prep_inputs(x, Wq1, bq1, Wq2, bq2, Wk1, bk1, Wk2, bk2):
    bf = ml_dtypes.bfloat16
    xT = np.ascontiguousarray(x.transpose(0, 2, 1)).astype(bf)  # [B,F,S]
    W1 = np.ascontiguousarray(np.stack([Wq1, Wk1])).astype(bf)  # [2,F,D]
    W2 = np.ascontiguousarray(
        np.stack([Wq2.reshape(2, 128, D), Wk2.reshape(2, 128, D)])).astype(bf)
    # per-partition const block: cols 0-3 = b1[s][m], 4-7 = b2[s][m2], 8 = -1
    BC = np.empty((128, 9), np.float32)
    for s, (b1v, b2v) in enumerate([(bq1, bq2), (bk1, bk2)]):
        for mm in range(2):
            BC[:, 2 * s + mm] = np.asarray(b1v)[mm * 128:(mm + 1) * 128]
            BC[:, 4 + 2 * s + mm] = np.asarray(b2v)[mm * 128:(mm + 1) * 128]
    BC[:, 8] = -1.0
    BC = np.ascontiguousarray(BC)
    return [
        {"xT": np.ascontiguousarray(xT[b]), "W1": W1, "W2": W2, "BC": BC}
        for b in range(B)
    ]


def _ensure_trace_hook():
    """Provide antenv.axon_hooks (NTFF profiling hook) if the image lacks it.

    Only matters when BASS_TRACE=1; degrades silently otherwise.
    """
    import sys
    import types
    try:
        import antenv.axon_hooks  # noqa: F401
        return
    except ImportError:
        pass
    try:
        import antenv
        from trn_agent_boot.trn_boot import _ntff_profile_via_ctypes

        mod = types.ModuleType("antenv.axon_hooks")
        state = {"hook": _ntff_profile_via_ctypes("/opt/axon/libaxon_pjrt.so")}
        mod.set_axon_ntff_profile_hook = lambda h: state.update(hook=h)
        mod.get_axon_ntff_profile_hook = lambda: state["hook"]
        sys.modules["antenv.axon_hooks"] = mod
        antenv.axon_hooks = mod
    except Exception:
        pass


def kernel(x, Wq1, bq1, Wq2, bq2, Wk1, bk1, Wk2, bk2):
    from concourse.bass_utils import run_bass_kernel_spmd

    try:
        _ensure_trace_hook()
    except Exception:
        pass

    nc = _get_nc()
    in_maps = _prep_inputs(x, Wq1, bq1, Wq2, bq2, Wk1, bk1, Wk2, bk2)
    res = run_bass_kernel_spmd(nc, in_maps, core_ids=list(range(NCORES)))
    _CACHED["last_results"] = res
    if res.exec_time_ns is not None:
        print(f"HW exec time: {res.exec_time_ns} ns")
    out = np.stack([res.results[b]["out"] for b in range(B)])
    # kernel computes/stores in bf16; deliver f32 to match the reference dtype
    return out.astype(np.float32)
